# revision 4
# baseline (speedup 1.0000x reference)
"""Trainium2 Bass kernel for the sparse_attention problem.

Math (per batch b, head h, with patch P = x[b] viewed as [C, N] (N=H*W)):
  QT[d,n] = q_w[h].T-projection of P (+bias), KT likewise        [HD, N]
  V[n,e]  = P.T @ v_w[h].T (bias folded into output)             [N, C]
  simT[k,q] = KT.T @ QT   (computed transposed: k on partitions)  [N, N]
  att logits[k] = sum_q simT[k,q] * scale  -> softmax over heads
  E = exp(simT*scale); En = E * (1/colsum) (softmax over k)
  generalT[e,q] = V.T @ En                                        [C, N]
  out[b,h,e,q] = generalT + att[h,q]*x[b,e,q] + v_b[h,e]

Distribution: pure data-parallel over batch, 2 batches per core on 8
NeuronCores, no collectives.  All matmuls in fp16 (full PE rate, ~4x the
mantissa of bf16; fp32 PSUM accumulation), softmax + epilogue in fp32.
"""

import math
from contextlib import ExitStack

import numpy as np

import concourse.bass as bass
import concourse.mybir as mybir
import concourse.tile as tile
from concourse import bacc
from concourse.masks import make_identity

B, C, H, W = 16, 2048, 16, 16
HEADS, HD = 4, 512
N = H * W                  # 256
NCORES = 8
BPC = B // NCORES          # batches per core = 2
SCALE = 1.0 / math.sqrt(HD)
CC = C // 128              # 16 channel chunks
KC = N // 128              # 2 key chunks
DC = HD // 128             # 4 head-dim chunks
NET = HEADS * C // 512     # 16 e-tiles for V projection

F16 = mybir.dt.float16
F32 = mybir.dt.float32
AX = mybir.AxisListType.X
AF = mybir.ActivationFunctionType


def build_bass():
    nc = bacc.Bacc(name="sparse_attention")

    # DRAM parameters; layouts chosen so every DMA below is a contiguous block.
    pt_d = nc.declare_dram_parameter("pt", [CC, 128, BPC * N], F16, False)
    xt_d = nc.declare_dram_parameter("xt", [BPC, CC, 128, N], F32, False)
    qkw_d = nc.declare_dram_parameter("qkw", [2 * HEADS, 2, CC, 128, 256], F16, False)
    qkb_d = nc.declare_dram_parameter("qkb", [128, 2 * HEADS * DC], F32, False)
    vw_d = nc.declare_dram_parameter("vw", [NET, CC, 128, 512], F16, False)
    vb_d = nc.declare_dram_parameter("vb", [128, HEADS * CC], F32, False)
    out_d = nc.declare_dram_parameter("out", [BPC, HEADS, CC, 128, N], F32, True)
    att_d = nc.declare_dram_parameter("att", [BPC, KC, 128, HEADS], F32, True)

    with tile.TileContext(nc) as tc:
        with ExitStack() as ctx:
            singles = ctx.enter_context(tc.tile_pool(name="singles", bufs=1))
            qkt_pool = ctx.enter_context(tc.tile_pool(name="qkt", bufs=1))
            vsb_pool = ctx.enter_context(tc.tile_pool(name="vsb", bufs=1))
            pt_pool = ctx.enter_context(tc.tile_pool(name="ptp", bufs=1))

            ident = singles.tile([128, 128], F32)
            make_identity(nc, ident)
            ones_k = singles.tile([128, 1], F16)
            nc.vector.memset(ones_k, 1.0)
            ones_1 = singles.tile([1, 128], F16)
            nc.vector.memset(ones_1, 1.0)
            qkb_sb = singles.tile([128, 2 * HEADS * DC], F32)
            nc.sync.dma_start(out=qkb_sb, in_=qkb_d[:, :])
            vb_sb = singles.tile([128, HEADS * CC], F32)
            nc.sync.dma_start(out=vb_sb, in_=vb_d[:, :])

            # resident activations / projections
            pt_sb = pt_pool.tile([128, CC, BPC * N], F16)      # patchT fp16
            for cc in range(CC):
                nc.sync.dma_start(out=pt_sb[:, cc, :], in_=pt_d[cc, :, :])
            qkt = qkt_pool.tile([128, 2 * HEADS * DC, BPC * N], F16)  # QT|KT
            vsb = vsb_pool.tile([128, BPC, KC, HEADS * C], F16)       # V

            # ---------------- Phase B: Q/K projections ----------------
            with tc.tile_pool(name="wqk", bufs=16) as wqk, \
                 tc.tile_pool(name="psB", bufs=4, space="PSUM") as psB:
                for s in range(2 * HEADS):          # 0..3 Q heads, 4..7 K heads
                    for dcp in range(2):            # pairs of 128-wide d chunks
                        wts = []
                        for cc in range(CC):
                            wt = wqk.tile([128, 256], F16, tag="wt")
                            nc.sync.dma_start(out=wt, in_=qkw_d[s, dcp, cc, :, :])
                            wts.append(wt)
                        for dh in range(2):
                            dc = dcp * 2 + dh
                            ps = psB.tile([128, BPC * N], F32, tag="psB")
                            for cc in range(CC):
                                nc.tensor.matmul(
                                    ps,
                                    lhsT=wts[cc][:, dh * 128:(dh + 1) * 128],
                                    rhs=pt_sb[:, cc, :],
                                    start=(cc == 0),
                                    stop=(cc == CC - 1),
                                )
                            # bias + cast to fp16 on the way out of PSUM
                            nc.scalar.activation(
                                qkt[:, s * DC + dc, :], ps, AF.Identity,
                                bias=qkb_sb[:, s * DC + dc:s * DC + dc + 1],
                            )

            # ---------------- Phase C: V projection ----------------
            with tc.tile_pool(name="wv", bufs=10) as wv, \
                 tc.tile_pool(name="psC", bufs=8, space="PSUM") as psC:
                for et in range(NET):
                    pss = {}
                    for b in range(BPC):
                        for kcc in range(KC):
                            pss[(b, kcc)] = psC.tile([128, 512], F32, tag="psC", name="psv")
                    for cc in range(CC):
                        vt = wv.tile([128, 512], F16, tag="vt")
                        nc.sync.dma_start(out=vt, in_=vw_d[et, cc, :, :])
                        for b in range(BPC):
                            for kcc in range(KC):
                                nc.tensor.matmul(
                                    pss[(b, kcc)],
                                    lhsT=pt_sb[:, cc, b * N + kcc * 128:b * N + (kcc + 1) * 128],
                                    rhs=vt,
                                    start=(cc == 0),
                                    stop=(cc == CC - 1),
                                )
                    for (b, kcc), ps in pss.items():
                        nc.scalar.copy(vsb[:, b, kcc, et * 512:(et + 1) * 512], ps)

            # ---------------- Phase D: attention + epilogue ----------------
            with tc.tile_pool(name="psDs", bufs=3, space="PSUM") as psDs, \
                 tc.tile_pool(name="psDm", bufs=2, space="PSUM") as psDm, \
                 tc.tile_pool(name="psDg", bufs=3, space="PSUM") as psDg, \
                 tc.tile_pool(name="enorm", bufs=12) as enorm, \
                 tc.tile_pool(name="attw", bufs=6) as attw, \
                 tc.tile_pool(name="xpool", bufs=4) as xpool, \
                 tc.tile_pool(name="outw", bufs=4) as outw:
                for b in range(BPC):
                    asum = attw.tile([128, KC, HEADS], F32, tag="asum")
                    ET = {}
                    EN = {}
                    for h in range(HEADS):
                        for kcc in range(KC):
                            ps_sim = psDs.tile([128, N], F32, tag="ps_sim")
                            for dc in range(DC):
                                nc.tensor.matmul(
                                    ps_sim,
                                    lhsT=qkt[:, (HEADS + h) * DC + dc,
                                             b * N + kcc * 128:b * N + (kcc + 1) * 128],
                                    rhs=qkt[:, h * DC + dc, b * N:(b + 1) * N],
                                    start=(dc == 0),
                                    stop=(dc == DC - 1),
                                )
                            nc.vector.reduce_sum(asum[:, kcc, h:h + 1], ps_sim, axis=AX)
                            et_t = enorm.tile([128, N], F16, tag="et")
                            nc.scalar.activation(et_t, ps_sim, AF.Exp, scale=SCALE)
                            ET[(h, kcc)] = et_t
                        # softmax denominator over k (partition dim) via ones-matmul
                        ps_rs = psDm.tile([1, N], F32, tag="ps_misc")
                        for kcc in range(KC):
                            nc.tensor.matmul(ps_rs, lhsT=ones_k, rhs=ET[(h, kcc)],
                                             start=(kcc == 0), stop=(kcc == KC - 1))
                        rs_sb = attw.tile([1, N], F32, tag="rs_sb")
                        nc.vector.reciprocal(rs_sb, ps_rs)
                        rinv16 = attw.tile([1, N], F16, tag="rinv16")
                        nc.scalar.copy(rinv16, rs_sb)
                        ps_rb = psDm.tile([128, N], F32, tag="ps_misc")
                        nc.tensor.matmul(ps_rb, lhsT=ones_1, rhs=rinv16)
                        rb = attw.tile([128, N], F16, tag="rb")
                        nc.scalar.copy(rb, ps_rb)
                        for kcc in range(KC):
                            en = enorm.tile([128, N], F16, tag="en")
                            nc.vector.tensor_mul(en, ET[(h, kcc)], rb)
                            EN[(h, kcc)] = en
                    # softmax over heads for attention_score
                    attTs = []
                    for kcc in range(KC):
                        m_t = attw.tile([128, 1], F32, tag="m_t")
                        nc.vector.reduce_max(m_t, asum[:, kcc, :], axis=AX)
                        nm = attw.tile([128, 1], F32, tag="nm")
                        nc.scalar.mul(nm, m_t, -SCALE)
                        ea = attw.tile([128, HEADS], F32, tag="ea")
                        nc.scalar.activation(ea, asum[:, kcc, :], AF.Exp,
                                             bias=nm, scale=SCALE)
                        sa = attw.tile([128, 1], F32, tag="sa")
                        nc.vector.reduce_sum(sa, ea, axis=AX)
                        ra = attw.tile([128, 1], F32, tag="ra")
                        nc.vector.reciprocal(ra, sa)
                        attT = attw.tile([128, HEADS], F32, tag="attT")
                        nc.vector.tensor_scalar_mul(attT, ea, ra)
                        nc.sync.dma_start(out=att_d[b, kcc, :, :], in_=attT)
                        attTs.append(attT)
                    # broadcast att rows to [128, N] tiles (transpose + ones-matmul)
                    AB = {}
                    for h in range(HEADS):
                        ps_row = psDm.tile([1, N], F32, tag="ps_misc", name="ps_row")
                        for kcc in range(KC):
                            nc.tensor.transpose(ps_row[:, kcc * 128:(kcc + 1) * 128],
                                                attTs[kcc][:, h:h + 1], ident)
                        row16 = attw.tile([1, N], F16, tag="row16")
                        nc.scalar.copy(row16, ps_row)
                        ps_ab = psDm.tile([128, N], F32, tag="ps_misc", name="ps_ab")
                        nc.tensor.matmul(ps_ab, lhsT=ones_1, rhs=row16)
                        ab = attw.tile([128, N], F32, tag="ab")
                        nc.scalar.copy(ab, ps_ab)
                        AB[h] = ab
                    # general attention + epilogue
                    for ec in range(CC):
                        xc = xpool.tile([128, N], F32, tag="xc")
                        nc.sync.dma_start(out=xc, in_=xt_d[b, ec, :, :])
                        for h in range(HEADS):
                            ps_g = psDg.tile([128, N], F32, tag="ps_g")
                            for kcc in range(KC):
                                nc.tensor.matmul(
                                    ps_g,
                                    lhsT=vsb[:, b, kcc, h * C + ec * 128:h * C + (ec + 1) * 128],
                                    rhs=EN[(h, kcc)],
                                    start=(kcc == 0),
                                    stop=(kcc == KC - 1),
                                )
                            sg = outw.tile([128, N], F32, tag="sg")
                            nc.scalar.activation(sg, ps_g, AF.Identity,
                                                 bias=vb_sb[:, h * CC + ec:h * CC + ec + 1])
                            tt = outw.tile([128, N], F32, tag="tt")
                            nc.vector.tensor_mul(tt, xc, AB[h])
                            oo = outw.tile([128, N], F32, tag="oo")
                            nc.vector.tensor_add(oo, sg, tt)
                            nc.sync.dma_start(out=out_d[b, h, ec, :, :], in_=oo)

    nc.compile()
    return nc


_NC_CACHE = {}


def _get_nc():
    if "nc" not in _NC_CACHE:
        _NC_CACHE["nc"] = build_bass()
    return _NC_CACHE["nc"]


def make_core_inputs(x, q_w, q_b, k_w, k_b, v_w, v_b):
    """Host-side prep: shard over batch, transpose weights, cast to fp16."""
    f16 = np.float16
    x = np.ascontiguousarray(np.asarray(x, dtype=np.float32))
    # weights (shared across cores)
    # qkw[s, dcp, cc, p, m]: s<4 -> Q head s, s>=4 -> K head s-4
    qk_w = np.concatenate([np.asarray(q_w), np.asarray(k_w)], axis=0)  # [8, HD, C]
    qkwT = qk_w.transpose(2, 0, 1).reshape(C, 2 * HEADS * HD)          # [c, (s d)]
    qkw = np.ascontiguousarray(
        qkwT.reshape(CC, 128, 2 * HEADS, 2, 256).transpose(2, 3, 0, 1, 4)
    ).astype(f16)
    qk_b = np.concatenate([np.asarray(q_b), np.asarray(k_b)], axis=0)  # [8, HD]
    qkb = np.ascontiguousarray(
        qk_b.reshape(2 * HEADS, DC, 128).transpose(2, 0, 1).reshape(128, 2 * HEADS * DC),
        dtype=np.float32)
    vwT = np.asarray(v_w).transpose(2, 0, 1).reshape(C, HEADS * C)     # [c, (h e)]
    vw = np.ascontiguousarray(
        vwT.reshape(CC, 128, NET, 512).transpose(2, 0, 1, 3)).astype(f16)
    vb = np.ascontiguousarray(
        np.asarray(v_b).reshape(HEADS, CC, 128).transpose(2, 0, 1).reshape(128, HEADS * CC),
        dtype=np.float32)

    in_maps = []
    for core in range(NCORES):
        xb = x[core * BPC:(core + 1) * BPC]                 # [BPC, C, H, W]
        patchT = xb.reshape(BPC, C, N)                      # [b, c, n]
        pt = np.ascontiguousarray(
            patchT.reshape(BPC, CC, 128, N).transpose(1, 2, 0, 3).reshape(CC, 128, BPC * N)
        ).astype(f16)
        xt = np.ascontiguousarray(patchT.reshape(BPC, CC, 128, N), dtype=np.float32)
        in_maps.append({
            "pt": pt, "xt": xt, "qkw": qkw, "qkb": qkb, "vw": vw, "vb": vb,
        })
    return in_maps


def assemble_outputs(results):
    """results: list of per-core dicts with 'out' [BPC,HEADS,CC,128,N] and
    'att' [BPC,KC,128,HEADS] -> full (attention_score, output)."""
    outs = []
    atts = []
    for r in results:
        o = np.asarray(r["out"], dtype=np.float32)
        a = np.asarray(r["att"], dtype=np.float32)
        outs.append(o.reshape(BPC, HEADS, C, N))
        atts.append(a.reshape(BPC, N, HEADS).transpose(0, 2, 1))  # [b, h, n]
    out_full = np.concatenate(outs, axis=0).reshape(B, HEADS * C, H, W)
    att_full = np.concatenate(atts, axis=0).reshape(B, HEADS, H, W)
    return att_full, out_full


def kernel(x, q_w, q_b, k_w, k_b, v_w, v_b):
    from concourse.bass_utils import run_bass_kernel_spmd

    nc = _get_nc()
    in_maps = make_core_inputs(x, q_w, q_b, k_w, k_b, v_w, v_b)
    res = run_bass_kernel_spmd(nc, in_maps, core_ids=list(range(NCORES)))
    return assemble_outputs(res.results)


# revision 22
# speedup vs baseline: 2.2887x; 2.2887x over previous
"""Trainium2 Bass kernel for the sparse_attention problem.

Math (per batch b, head h, with patch P = x[b] viewed as [C, N] (N=H*W)):
  QT[d,n] = q_w[h].T-projection of P (+bias), KT likewise        [HD, N]
  V[n,e]  = P.T @ v_w[h].T (bias folded into output)             [N, C]
  simT[k,q] = KT.T @ QT   (computed transposed: k on partitions)  [N, N]
  att logits[k] = sum_q simT[k,q] * scale  -> softmax over heads
  E = exp(simT*scale); En = E * (1/colsum) (softmax over k)
  generalT[e,q] = V.T @ En                                        [C, N]
  out[b,h,e,q] = generalT + att[h,q]*x[b,e,q] + v_b[h,e]

Distribution: pure data-parallel over batch, 2 batches per core on 8
NeuronCores, no collectives.  All matmuls in fp16 (full PE rate, ~4x the
mantissa of bf16; fp32 PSUM accumulation), softmax + epilogue in fp32.

Schedule: phase B (Q/K proj) -> phase S (sim, softmaxes, att broadcast)
-> phase C' (V proj fused with general matmul + epilogue per e-tile).
All input DMAs are single contiguous 256-512KB blocks; weight loads ride
the sync HWDGE ring, x/out the scalar ring.
"""

import math
from contextlib import ExitStack

import numpy as np

import concourse.bass as bass
import concourse.mybir as mybir
import concourse.tile as tile
from concourse import bacc
from concourse.masks import make_identity

B, C, H, W = 16, 2048, 16, 16
HEADS, HD = 4, 512
N = H * W                  # 256
NCORES = 8
BPC = B // NCORES          # batches per core = 2
SCALE = 1.0 / math.sqrt(HD)
CC = C // 128              # 16 channel chunks
KC = N // 128              # 2 key chunks
DC = HD // 128             # 4 head-dim chunks
NEQ = 4                    # e-quads per head (C/512)

F16 = mybir.dt.float16
F32 = mybir.dt.float32
F8 = mybir.dt.float8e4
AX = mybir.AxisListType.X
AF = mybir.ActivationFunctionType

FP8_V = True   # fp8 DoubleRow V projection (2x PE rate, ~7e-3 rel err vs 1e-3)


def build_bass():
    nc = bacc.Bacc(name="sparse_attention")

    # DRAM parameters; layouts chosen so every DMA below is one contiguous block.
    pt_d = nc.declare_dram_parameter("pt", [4, 128, 4, BPC * N], F16, False)
    xt_d = nc.declare_dram_parameter("xt", [BPC, NEQ, 128, 4, N], F32, False)
    qkw_d = nc.declare_dram_parameter("qkw", [2 * HEADS, 2, 4, 128, 4, 256], F16, False)
    qkb_d = nc.declare_dram_parameter("qkb", [128, 2 * HEADS * DC], F32, False)
    if FP8_V:
        # vw8[et, ccq, p, j2, i, m]: contraction c = (ccq*4 + j2*2 + i)*128 + p
        vw_d = nc.declare_dram_parameter("vw", [HEADS * NEQ, 4, 128, 2, 2, 512], F8, False)
        # pt8[half, p, cp4, i, m]: c = ((half*4+cp4)*2 + i)*128 + p, m = b*N+q
        pt8_d = nc.declare_dram_parameter("pt8", [2, 128, 4, 2, BPC * N], F8, False)
    else:
        vw_d = nc.declare_dram_parameter("vw", [HEADS * NEQ, 4, 128, 4, 512], F16, False)
    vb_d = nc.declare_dram_parameter("vb", [1, HEADS * C], F16, False)
    out_d = nc.declare_dram_parameter("out", [BPC, HEADS, NEQ, 128, 4, N], F32, True)
    att_d = nc.declare_dram_parameter("att", [BPC, KC, 128, HEADS], F32, True)

    with tile.TileContext(nc) as tc:
        with ExitStack() as ctx:
            singles = ctx.enter_context(tc.tile_pool(name="singles", bufs=1))
            vsb_pool = ctx.enter_context(tc.tile_pool(name="vsb", bufs=1))
            pt_pool = ctx.enter_context(tc.tile_pool(name="ptp", bufs=1))
            en_pool = ctx.enter_context(tc.tile_pool(name="enp", bufs=16))
            ab_pool = ctx.enter_context(tc.tile_pool(name="abp", bufs=8))

            ident = singles.tile([128, 128], F32)
            make_identity(nc, ident)
            ones_k = singles.tile([128, 1], F16)
            nc.vector.memset(ones_k, 1.0)
            ones_1 = singles.tile([1, 128], F16)
            nc.vector.memset(ones_1, 1.0)
            qkb_sb = singles.tile([128, 2 * HEADS * DC], F32)
            nc.sync.dma_start(out=qkb_sb, in_=qkb_d[:, :])
            vb_sb = singles.tile([1, HEADS * C], F16)

            pt_sb = pt_pool.tile([128, CC, BPC * N], F16)      # patchT fp16
            for ccq in range(4):
                eng = nc.sync if ccq % 2 == 0 else nc.scalar
                eng.dma_start(out=pt_sb[:, ccq * 4:(ccq + 1) * 4, :],
                              in_=pt_d[ccq, :, :, :])
            vsb = vsb_pool.tile([128, BPC, KC, HEADS * C], F16)  # V resident

            EN = {}
            AB = {}
            # C'-phase pools open first (LIFO: they release last) so
            # V-projection matmuls can fill PE idle time during phase S.
            cprime = ExitStack()
            wv = cprime.enter_context(tc.tile_pool(name="wv", bufs=6))
            xp = cprime.enter_context(tc.tile_pool(name="xp", bufs=3))
            ow = cprime.enter_context(tc.tile_pool(name="ow", bufs=3))
            sw = cprime.enter_context(tc.tile_pool(name="sw", bufs=4))
            psC = cprime.enter_context(tc.tile_pool(name="psC", bufs=6, space="PSUM"))
            with tc.tile_pool(name="qkt", bufs=1) as qkt_pool:
                qkt = qkt_pool.tile([128, 2 * HEADS * DC, BPC * N], F16)

                # ---------------- Phase B: Q/K projections ----------------
                with tc.tile_pool(name="wqk", bufs=5) as wqk, \
                     tc.tile_pool(name="psB", bufs=2, space="PSUM") as psB:
                    for s in range(2 * HEADS):     # 0..3 Q heads, 4..7 K heads
                        for dcp in range(2):       # pairs of 128-wide d chunks
                            wts = []
                            for ccq in range(4):
                                wt = wqk.tile([128, 4, 256], F16, tag="wt")
                                nc.sync.dma_start(out=wt, in_=qkw_d[s, dcp, ccq, :, :, :])
                                wts.append(wt)
                            for dh in range(2):
                                dc = dcp * 2 + dh
                                ps = psB.tile([128, BPC * N], F32, tag="psB")
                                for cc in range(CC):
                                    nc.tensor.matmul(
                                        ps,
                                        lhsT=wts[cc // 4][:, cc % 4, dh * 128:(dh + 1) * 128],
                                        rhs=pt_sb[:, cc, :],
                                        start=(cc == 0),
                                        stop=(cc == CC - 1),
                                    )
                                nc.scalar.activation(
                                    qkt[:, s * DC + dc, :], ps, AF.Identity,
                                    bias=qkb_sb[:, s * DC + dc:s * DC + dc + 1],
                                )

                # deferred loads (needed only from phase C'): off the startup path
                nc.scalar.dma_start(out=vb_sb, in_=vb_d[:, :])
                if FP8_V:
                    pt8_sb = pt_pool.tile([128, 8, 2, BPC * N], F8)
                    for hf in range(2):
                        nc.scalar.dma_start(out=pt8_sb[:, hf * 4:(hf + 1) * 4, :, :],
                                            in_=pt8_d[hf, :, :, :, :])

                # ------ Phase S: sim, both softmaxes, att broadcast ------
                with tc.tile_pool(name="psSs", bufs=1, space="PSUM") as psSs, \
                     tc.tile_pool(name="psSm", bufs=1, space="PSUM") as psSm, \
                     tc.tile_pool(name="attw", bufs=6) as attw:
                    for b in range(BPC):
                        asum = attw.tile([128, KC, HEADS], F32, tag="asum", bufs=2)
                        ET = {}
                        for h in range(HEADS):
                            for kcc in range(KC):
                                ps_sim = psSs.tile([128, N], F32, tag="ps_sim")
                                for dc in range(DC):
                                    nc.tensor.matmul(
                                        ps_sim,
                                        lhsT=qkt[:, (HEADS + h) * DC + dc,
                                                 b * N + kcc * 128:b * N + (kcc + 1) * 128],
                                        rhs=qkt[:, h * DC + dc, b * N:(b + 1) * N],
                                        start=(dc == 0),
                                        stop=(dc == DC - 1),
                                    )
                                nc.vector.reduce_sum(asum[:, kcc, h:h + 1], ps_sim, axis=AX)
                                et_t = attw.tile([128, N], F16, tag="et", bufs=5)
                                nc.scalar.activation(et_t, ps_sim, AF.Exp, scale=SCALE)
                                ET[(h, kcc)] = et_t
                            # softmax denominator over k (partition dim) via ones-mm
                            ps_rs = psSm.tile([1, N], F32, tag="ps_misc")
                            for kcc in range(KC):
                                nc.tensor.matmul(ps_rs, lhsT=ones_k, rhs=ET[(h, kcc)],
                                                 start=(kcc == 0), stop=(kcc == KC - 1))
                            rs_sb = attw.tile([1, N], F32, tag="rs_sb", bufs=2)
                            nc.vector.reciprocal(rs_sb, ps_rs)
                            rinv16 = attw.tile([1, N], F16, tag="rinv16", bufs=2)
                            nc.scalar.copy(rinv16, rs_sb)
                            ps_rb = psSm.tile([128, N], F32, tag="ps_misc")
                            nc.tensor.matmul(ps_rb, lhsT=ones_1, rhs=rinv16)
                            rb = attw.tile([128, N], F16, tag="rb", bufs=3)
                            nc.scalar.copy(rb, ps_rb)
                            for kcc in range(KC):
                                en = en_pool.tile([128, N], F16, tag="en")
                                nc.vector.tensor_mul(en, ET[(h, kcc)], rb)
                                EN[(b, h, kcc)] = en
                        # softmax over heads for attention_score
                        attTs = []
                        for kcc in range(KC):
                            m_t = attw.tile([128, 1], F32, tag="m_t")
                            nc.vector.reduce_max(m_t, asum[:, kcc, :], axis=AX)
                            nm = attw.tile([128, 1], F32, tag="nm")
                            nc.scalar.mul(nm, m_t, -SCALE)
                            ea = attw.tile([128, HEADS], F32, tag="ea")
                            nc.scalar.activation(ea, asum[:, kcc, :], AF.Exp,
                                                 bias=nm, scale=SCALE)
                            sa = attw.tile([128, 1], F32, tag="sa")
                            nc.vector.reduce_sum(sa, ea, axis=AX)
                            ra = attw.tile([128, 1], F32, tag="ra")
                            nc.vector.reciprocal(ra, sa)
                            attT = attw.tile([128, HEADS], F32, tag="attT")
                            nc.vector.tensor_scalar_mul(attT, ea, ra)
                            nc.scalar.dma_start(out=att_d[b, kcc, :, :], in_=attT)
                            attTs.append(attT)
                        # broadcast att rows to [128, N] tiles
                        for h in range(HEADS):
                            ps_row = psSm.tile([1, N], F32, tag="ps_misc", name="ps_row")
                            for kcc in range(KC):
                                nc.tensor.transpose(ps_row[:, kcc * 128:(kcc + 1) * 128],
                                                    attTs[kcc][:, h:h + 1], ident)
                            row16 = attw.tile([1, N], F16, tag="row16", bufs=2)
                            nc.scalar.copy(row16, ps_row)
                            ps_ab = psSm.tile([128, N], F32, tag="ps_misc", name="ps_ab")
                            nc.tensor.matmul(ps_ab, lhsT=ones_1, rhs=row16)
                            ab = ab_pool.tile([128, N], F32, tag="ab")
                            nc.scalar.copy(ab, ps_ab)
                            AB[(b, h)] = ab

            # -------- Phase C': V proj fused with general + epilogue --------
            with cprime, tc.tile_pool(name="psG", bufs=2, space="PSUM") as psG:
                for eq in range(NEQ):
                    xcs = {}
                    for b in range(BPC):
                        xc = xp.tile([128, 4, N], F32, tag="xc")
                        nc.scalar.dma_start(out=xc, in_=xt_d[b, eq, :, :, :])
                        xcs[b] = xc
                    for h in range(HEADS):
                        et = h * NEQ + eq
                        # V projection for this e-tile (both batches)
                        pss = {}
                        for b in range(BPC):
                            for kcc in range(KC):
                                pss[(b, kcc)] = psC.tile([128, 512], F32,
                                                         tag="psC", name="psv")
                        if FP8_V:
                            vts = []
                            for ccq in range(4):
                                vt = wv.tile([128, 2, 2, 512], F8, tag="vt")
                                nc.sync.dma_start(out=vt, in_=vw_d[et, ccq, :, :, :, :])
                                vts.append(vt)
                            for ccp in range(8):
                                for b in range(BPC):
                                    for kcc in range(KC):
                                        nc.tensor.matmul(
                                            pss[(b, kcc)],
                                            lhsT=pt8_sb[:, ccp, :, b * N + kcc * 128:b * N + (kcc + 1) * 128],
                                            rhs=vts[ccp // 2][:, ccp % 2, :, :],
                                            start=(ccp == 0),
                                            stop=False,
                                            perf_mode=mybir.MatmulPerfMode.DoubleRow,
                                        )
                            # + ones x v_b row: V += 1 (x) vb  (softmax rows sum
                            # to 1, so this lands the output bias exactly)
                            for b in range(BPC):
                                for kcc in range(KC):
                                    nc.tensor.matmul(
                                        pss[(b, kcc)], lhsT=ones_1,
                                        rhs=vb_sb[0:1, et * 512:(et + 1) * 512],
                                        start=False, stop=True)
                        else:
                            vts = []
                            for ccq in range(4):
                                vt = wv.tile([128, 4, 512], F16, tag="vt")
                                nc.sync.dma_start(out=vt, in_=vw_d[et, ccq, :, :, :])
                                vts.append(vt)
                            for cc in range(CC):
                                for b in range(BPC):
                                    for kcc in range(KC):
                                        nc.tensor.matmul(
                                            pss[(b, kcc)],
                                            lhsT=pt_sb[:, cc, b * N + kcc * 128:b * N + (kcc + 1) * 128],
                                            rhs=vts[cc // 4][:, cc % 4, :],
                                            start=(cc == 0),
                                            stop=False,
                                        )
                            for b in range(BPC):
                                for kcc in range(KC):
                                    nc.tensor.matmul(
                                        pss[(b, kcc)], lhsT=ones_1,
                                        rhs=vb_sb[0:1, et * 512:(et + 1) * 512],
                                        start=False, stop=True)
                        for (b, kcc), ps in pss.items():
                            dst = vsb[:, b, kcc, et * 512:(et + 1) * 512]
                            if b == 0:
                                nc.scalar.copy(dst, ps)
                            else:
                                nc.vector.tensor_copy(dst, ps)
                        # general attention + epilogue for these 4 e-chunks
                        for b in range(BPC):
                            oo = ow.tile([128, 4, N], F32, tag="oo")
                            for e4 in range(4):
                                ec = eq * 4 + e4
                                ps_g = psG.tile([128, N], F32, tag="ps_g")
                                for kcc in range(KC):
                                    nc.tensor.matmul(
                                        ps_g,
                                        lhsT=vsb[:, b, kcc, h * C + ec * 128:h * C + (ec + 1) * 128],
                                        rhs=EN[(b, h, kcc)],
                                        start=(kcc == 0),
                                        stop=(kcc == KC - 1),
                                    )
                                tt = sw.tile([128, N], F32, tag="tt")
                                nc.gpsimd.tensor_mul(tt, xcs[b][:, e4, :], AB[(b, h)])
                                nc.vector.tensor_add(oo[:, e4, :], ps_g, tt)
                            nc.scalar.dma_start(out=out_d[b, h, eq, :, :, :], in_=oo)

    nc.compile()
    return nc


_NC_CACHE = {}


def _get_nc():
    if "nc" not in _NC_CACHE:
        _NC_CACHE["nc"] = build_bass()
    return _NC_CACHE["nc"]


def make_core_inputs(x, q_w, q_b, k_w, k_b, v_w, v_b):
    """Host-side prep: shard over batch, transpose weights, cast to fp16."""
    f16 = np.float16
    x = np.ascontiguousarray(np.asarray(x, dtype=np.float32))
    # qkw[s, dcp, ccq, p, c4, m]: s<4 -> Q head s, s>=4 -> K head s-4
    qk_w = np.concatenate([np.asarray(q_w), np.asarray(k_w)], axis=0)  # [8, HD, C]
    qkwT = qk_w.transpose(2, 0, 1).reshape(C, 2 * HEADS * HD)          # [c, (s d)]
    qkw = np.ascontiguousarray(
        qkwT.reshape(4, 4, 128, 2 * HEADS, 2, 256).transpose(3, 4, 0, 2, 1, 5)
    ).astype(f16)
    qk_b = np.concatenate([np.asarray(q_b), np.asarray(k_b)], axis=0)  # [8, HD]
    qkb = np.ascontiguousarray(
        qk_b.reshape(2 * HEADS, DC, 128).transpose(2, 0, 1).reshape(128, 2 * HEADS * DC),
        dtype=np.float32)
    vwT = np.asarray(v_w).transpose(2, 0, 1).reshape(C, HEADS * C)     # [c, (h e)]
    if FP8_V:
        import ml_dtypes
        f8 = ml_dtypes.float8_e4m3
        # vw8[et, ccq, p, j2, i, m]: c = (ccq*4 + j2*2 + i)*128 + p
        vw = np.ascontiguousarray(
            vwT.reshape(4, 2, 2, 128, HEADS * NEQ, 512)
            .transpose(4, 0, 3, 1, 2, 5)).astype(f8)
    else:
        vw = np.ascontiguousarray(
            vwT.reshape(4, 4, 128, HEADS * NEQ, 512).transpose(3, 0, 2, 1, 4)).astype(f16)
    vb = np.ascontiguousarray(np.asarray(v_b).reshape(1, HEADS * C)).astype(f16)

    in_maps = []
    for core in range(NCORES):
        xb = x[core * BPC:(core + 1) * BPC]                 # [BPC, C, H, W]
        patchT = xb.reshape(BPC, C, N)                      # [b, c, n]
        # pt[ccq, p, c4, b*N+q]
        pt = np.ascontiguousarray(
            patchT.reshape(BPC, 4, 4, 128, N).transpose(1, 3, 2, 0, 4)
            .reshape(4, 128, 4, BPC * N)).astype(f16)
        # xt[b, eq, p, e4, q]
        xt = np.ascontiguousarray(
            patchT.reshape(BPC, NEQ, 4, 128, N).transpose(0, 1, 3, 2, 4),
            dtype=np.float32)
        m = {"pt": pt, "xt": xt, "qkw": qkw, "qkb": qkb, "vw": vw, "vb": vb}
        if FP8_V:
            import ml_dtypes
            f8 = ml_dtypes.float8_e4m3
            # pt8[half, p, cp4, i, b*N+q]: c = ((half*4+cp4)*2 + i)*128 + p
            m["pt8"] = np.ascontiguousarray(
                patchT.reshape(BPC, 2, 4, 2, 128, N).transpose(1, 4, 2, 3, 0, 5)
                .reshape(2, 128, 4, 2, BPC * N)).astype(f8)
        in_maps.append(m)
    return in_maps


def assemble_outputs(results):
    """results: per-core dicts with 'out' [BPC,HEADS,NEQ,128,4,N] and
    'att' [BPC,KC,128,HEADS] -> full (attention_score, output)."""
    outs = []
    atts = []
    for r in results:
        o = np.asarray(r["out"], dtype=np.float32)
        a = np.asarray(r["att"], dtype=np.float32)
        # (b,h,eq,p,e4,q) -> (b,h,eq,e4,p,q) -> [b,h,C,N]
        outs.append(o.transpose(0, 1, 2, 4, 3, 5).reshape(BPC, HEADS, C, N))
        atts.append(a.reshape(BPC, N, HEADS).transpose(0, 2, 1))  # [b, h, n]
    out_full = np.concatenate(outs, axis=0).reshape(B, HEADS * C, H, W)
    att_full = np.concatenate(atts, axis=0).reshape(B, HEADS, H, W)
    return att_full, out_full


def kernel(x, q_w, q_b, k_w, k_b, v_w, v_b):
    from concourse.bass_utils import run_bass_kernel_spmd

    nc = _get_nc()
    in_maps = make_core_inputs(x, q_w, q_b, k_w, k_b, v_w, v_b)
    res = run_bass_kernel_spmd(nc, in_maps, core_ids=list(range(NCORES)))
    return assemble_outputs(res.results)
